# revision 2
# baseline (speedup 1.0000x reference)
"""Trainium2 Bass kernel for nn_AMLNeuralNetwork3D (dense_mlp).

Strategy: 8-way tensor parallel (column split on output features) for all
three 8192x8192 dense layers; the per-gene local layer shards along the
gene axis (matching the feature split).  After the local layer and after
L1/L2 the per-core feature slices are AllGather'd (concat on partition
axis = gene axis).  L3 slices are returned per-core and assembled on host.

Layout: activations are kept feature-major [features, batch] on chip so a
layer's output layout equals the next layer's input layout (contraction is
over the partition axis on the TensorEngine).  Weights are pre-transposed
on host to [in_features, out_slice] so all DMAs are wide/contiguous.

Prologue structure (the critical path before L1 can start):
 - a dummy 4KB AllGather is emitted first so the CC-channel init /
   launch barrier completes while the x DMA + local layer run;
 - the local layer computes on full-batch [128,1024] tiles (contiguous
   256KB x DMAs) producing both batch-chunks' outputs at once;
 - the first-transition AllGather is split into 4 feature QUARTERS per
   batch chunk; L1 starts as soon as quarter 0 lands (w1t rows are
   host-permuted to quarter-major gather order).

Compute in bf16 (full-rate on the PE, fp32 PSUM accumulation); measured
L2 rel-err of the full net in bf16 is ~5e-3.
"""

import sys

if "/opt/trn_rl_repo" not in sys.path:
    sys.path.insert(0, "/opt/trn_rl_repo")

import numpy as np
import ml_dtypes

N_CORES = 8
G = 8192          # genes / features
B = 1024          # batch
L = 4             # levels
GS = G // N_CORES # per-core feature slice (1024)
NB = 512          # batch chunk (one PSUM bank at fp32)
NCHUNK = B // NB  # 2
GT = GS // 128    # gene tiles per core slice (8)
KT = G // 128     # contraction tiles (64)
NQ = 4            # feature quarters for the first AllGather transition
QROWS = GS // NQ  # 256 rows per quarter slice

BF16 = ml_dtypes.bfloat16

_compiled = {}

# gathered-feature order for the quarter-split first AllGather:
# [core0 f0:256, core1 f1024:1280, ..., core7, then second quarters, ...]
_PERM_QUARTERS = np.concatenate(
    [np.arange(r * GS + a * QROWS, r * GS + (a + 1) * QROWS)
     for a in range(NQ) for r in range(N_CORES)]
)

N_WARMUP = 76


def _build_graph():
    from concourse import bacc, tile
    from concourse.tile_rust import add_dep_helper
    import concourse.mybir as mybir

    fp32 = mybir.dt.float32
    bf16 = mybir.dt.bfloat16
    Relu = mybir.ActivationFunctionType.Relu
    mult = mybir.AluOpType.mult
    add = mybir.AluOpType.add
    bypass = mybir.AluOpType.bypass

    nc = bacc.Bacc(None, target_bir_lowering=False, num_devices=N_CORES)

    # ---- parameters (per-core shards; same graph on all cores) ----
    x_p = nc.declare_dram_parameter("x", [L, GS, B], bf16, isOutput=False)
    # per-feature scalars: cols 0..3 = W_local, 4 = b_local, 5..7 = b1..b3
    scal_p = nc.declare_dram_parameter("scal", [GS, 8], fp32, isOutput=False)
    w_p = [
        nc.declare_dram_parameter(f"w{k}t", [G, GS], bf16, isOutput=False)
        for k in (1, 2, 3)
    ]
    out_p = nc.declare_dram_parameter("out", [GS, B], fp32, isOutput=True)

    rg = [list(range(N_CORES))]

    with tile.TileContext(nc) as tc:
        with (
            tc.tile_pool(name="dram", bufs=1, space="DRAM") as dram,
            tc.tile_pool(name="scal", bufs=GT) as spool,
            tc.tile_pool(name="xin", bufs=10) as xpool,
            tc.tile_pool(name="loc", bufs=6) as lpool,
            tc.tile_pool(name="h0p", bufs=GT) as h0pool,
            tc.tile_pool(name="hin", bufs=32) as hpool,
            tc.tile_pool(name="wblk", bufs=24) as wpool,
            tc.tile_pool(name="hout", bufs=6) as opool,
            tc.tile_pool(name="psum", bufs=8, space="PSUM") as ppool,
        ):
            _gath_space = "Shared"
            # --- dummy AllGather: rides through the CC-channel init /
            # launch barrier while x DMA + local layer run, so the real
            # quarter-gathers below start transferring immediately.
            dummy_in = dram.tile([128, 8], fp32, name="dummy_in", tag="dmi")
            nc.sync.dma_start(dummy_in[:], scal_p[0:128, :])
            dummy_out = dram.tile(
                [128 * N_CORES, 8], fp32, name="dummy_out", tag="dmo",
                addr_space=_gath_space,
            )
            nc.gpsimd.collective_compute(
                "AllGather", bypass, replica_groups=rg,
                ins=[dummy_in[:].opt()], outs=[dummy_out[:].opt()],
            )

            # bounce buffers: first transition is quarter-split per chunk
            slc0q = [
                [
                    dram.tile([QROWS, NB], bf16, name=f"slc0q{j}_{a}",
                              tag=f"slc0q{j}_{a}")
                    for a in range(NQ)
                ]
                for j in range(NCHUNK)
            ]
            gath0q = [
                [
                    dram.tile(
                        [QROWS * N_CORES, NB], bf16, name=f"gath0q{j}_{a}",
                        tag=f"gath0q{j}_{a}", addr_space=_gath_space,
                    )
                    for a in range(NQ)
                ]
                for j in range(NCHUNK)
            ]
            # transitions 1,2 (after L1/L2): one AG per batch chunk
            slc = [
                [
                    dram.tile([GS, NB], bf16, name=f"slc_{t}_{j}", tag=f"slc_{t}_{j}")
                    for j in range(NCHUNK)
                ]
                for t in range(1, 3)
            ]
            gath = [
                [
                    dram.tile(
                        [G, NB], bf16, name=f"gath_{t}_{j}", tag=f"gath_{t}_{j}",
                        addr_space=_gath_space,
                    )
                    for j in range(NCHUNK)
                ]
                for t in range(1, 3)
            ]

            # --- PE warmup: keeps the HAM clock-gate warm through the
            # prologue (x DMA + local layer + first quarter-gathers).
            wu_w = spool.tile([128, 128], bf16, name="wu_w", tag="wu_w")
            nc.sync.dma_start(wu_w[:], w_p[0][0:128, 0:128])
            wu_h = spool.tile([128, NB], bf16, name="wu_h", tag="wu_h")
            nc.sync.dma_start(wu_h[:], w_p[0][0:128, 0:NB])
            wu_ps = ppool.tile([128, NB], fp32, name="wu_ps", tag="ps")
            for i in range(N_WARMUP):
                nc.tensor.matmul(
                    wu_ps[:], wu_w[:], wu_h[:],
                    start=(i == 0), stop=(i == N_WARMUP - 1),
                )
            wu_out = spool.tile([128, NB], bf16, name="wu_out", tag="wu_out")
            nc.scalar.activation(
                wu_out[:], wu_ps[:], mybir.ActivationFunctionType.Copy
            )
            wu_dram = dram.tile([128, NB], bf16, name="wu_dram", tag="wu_dram")
            nc.scalar.dma_start(wu_dram[:], wu_out[:])

            # per-feature scalar tiles, persistent
            sc = []
            for gt in range(GT):
                s = spool.tile([128, 8], fp32, name=f"sc{gt}", tag="sc")
                nc.sync.dma_start(s[:], scal_p[gt * 128 : (gt + 1) * 128, :])
                sc.append(s)

            # ---- local layer on full-batch [128, B] tiles (both chunks) ----
            # x DMAs go on the sync ring (no gated entries before them);
            # activations + slc writes go via the scalar queue so the ring
            # never head-of-line blocks on compute.
            acts = []
            for gt in range(GT):
                xt = []
                for l in range(L):
                    t = xpool.tile([128, B], bf16, name=f"x{gt}_{l}", tag="x")
                    nc.sync.dma_start(t[:], x_p[l, gt * 128 : (gt + 1) * 128, :])
                    xt.append(t)
                acc = lpool.tile([128, B], bf16, name=f"a{gt}_0", tag="acc")
                nc.vector.tensor_scalar(
                    acc[:], xt[0][:], sc[gt][:, 0:1], None, mult
                )
                for l in range(1, L):
                    acc2 = lpool.tile([128, B], bf16, name=f"a{gt}_{l}", tag="acc")
                    nc.vector.scalar_tensor_tensor(
                        acc2[:], xt[l][:], sc[gt][:, l : l + 1], acc[:], mult, add
                    )
                    acc = acc2
                h0 = h0pool.tile([128, B], bf16, name=f"h0_{gt}", tag="h0")
                ai = nc.scalar.activation(h0[:], acc[:], Relu, bias=sc[gt][:, 4:5])
                acts.append(ai)
                a, row = gt // 2, (gt % 2) * 128
                for j in range(NCHUNK):
                    nc.scalar.dma_start(
                        slc0q[j][a][row : row + 128, :],
                        h0[:, j * NB : (j + 1) * NB],
                    )

            # quarter-gathers: chunk 0's four quarters first (they gate L1),
            # then chunk 1's
            for j in range(NCHUNK):
                for a in range(NQ):
                    nc.gpsimd.collective_compute(
                        "AllGather", bypass, replica_groups=rg,
                        ins=[slc0q[j][a][:].opt()], outs=[gath0q[j][a][:].opt()],
                    )

            def dense_layer(k, j):
                # k in {1,2,3}; input from gath0q[j] (k==1) or gath[k-2][j];
                # output slice -> slc[k-1][j] (k<3) or out_p (k==3)
                wt = w_p[k - 1]
                ps = [
                    ppool.tile([128, NB], fp32, name=f"ps{k}_{j}_{o}", tag="ps")
                    for o in range(GT)
                ]
                # prefetch the first W blocks ahead of the h stream; for L1
                # chunk 0 hold them behind the local layer's x tiles
                wdmas = {}
                NPREF = 8
                for g in range(NPREF):
                    wb = wpool.tile([128, GS], bf16, name=f"w{k}_{j}_{g}", tag="wblk")
                    wdma = nc.sync.dma_start(wb[:], wt[g * 128 : (g + 1) * 128, :])
                    if k == 1 and j == 0:
                        add_dep_helper(
                            getattr(wdma, "ins", wdma),
                            getattr(acts[3], "ins", acts[3]),
                            reason="x tiles first on HBM",
                        )
                    wdmas[g] = wb
                for g in range(KT):
                    ht = hpool.tile([128, NB], bf16, name=f"h{k}_{j}_{g}", tag="hin")
                    if k == 1:
                        hsrc = gath0q[j][g // (KT // NQ)]
                        row = (g % (KT // NQ)) * 128
                        nc.sync.dma_start(ht[:], hsrc[row : row + 128, :])
                    else:
                        src = gath[k - 2][j]
                        nc.sync.dma_start(ht[:], src[g * 128 : (g + 1) * 128, :])
                    if g + NPREF < KT:
                        gq = g + NPREF
                        wb = wpool.tile(
                            [128, GS], bf16, name=f"w{k}_{j}_{gq}", tag="wblk"
                        )
                        nc.sync.dma_start(wb[:], wt[gq * 128 : (gq + 1) * 128, :])
                        wdmas[gq] = wb
                    wb = wdmas.pop(g)
                    for o in range(GT):
                        nc.tensor.matmul(
                            ps[o][:],
                            wb[:, o * 128 : (o + 1) * 128],
                            ht[:],
                            start=(g == 0),
                            stop=(g == KT - 1),
                        )
                for o in range(GT):
                    if k < 3:
                        ot = opool.tile(
                            [128, NB], bf16, name=f"o{k}_{j}_{o}", tag="hout"
                        )
                        nc.scalar.activation(
                            ot[:], ps[o][:], Relu, bias=sc[o][:, 4 + k : 5 + k]
                        )
                        nc.sync.dma_start(
                            slc[k - 1][j][o * 128 : (o + 1) * 128, :], ot[:]
                        )
                    else:
                        ot = opool.tile(
                            [128, NB], fp32, name=f"o{k}_{j}_{o}", tag="outp"
                        )
                        nc.scalar.activation(
                            ot[:], ps[o][:], Relu, bias=sc[o][:, 7:8]
                        )
                        nc.sync.dma_start(
                            out_p[o * 128 : (o + 1) * 128, j * NB : (j + 1) * NB],
                            ot[:],
                        )

            def allgather(t, j):
                nc.gpsimd.collective_compute(
                    "AllGather",
                    bypass,
                    replica_groups=rg,
                    ins=[slc[t - 1][j][:].opt()],
                    outs=[gath[t - 1][j][:].opt()],
                )

            # emission order = desired overlap order
            for k in (1, 2, 3):
                for j in range(NCHUNK):
                    dense_layer(k, j)
                    if k < 3:
                        allgather(k, j)

    nc.compile()
    return nc


def _get_nc():
    if "nc" not in _compiled:
        _compiled["nc"] = _build_graph()
    return _compiled["nc"]


def kernel(x, W_local, b_local, W1, b1, W2, b2, W3, b3):
    from concourse.bass_utils import run_bass_kernel_spmd

    nc = _get_nc()

    x = np.asarray(x)
    in_maps = []
    for r in range(N_CORES):
        sl = slice(r * GS, (r + 1) * GS)
        x_r = x[:, :, sl].transpose(0, 2, 1).astype(BF16)
        scal_r = np.concatenate(
            [
                np.asarray(W_local)[sl, :],
                np.asarray(b_local)[sl, None],
                np.asarray(b1)[sl, None],
                np.asarray(b2)[sl, None],
                np.asarray(b3)[sl, None],
            ],
            axis=1,
        ).astype(np.float32)
        in_maps.append(
            {
                "x": x_r,
                "scal": np.ascontiguousarray(scal_r),
                "w1t": np.asarray(W1)[sl, :].T.astype(BF16)[_PERM_QUARTERS, :],
                "w2t": np.asarray(W2)[sl, :].T.astype(BF16),
                "w3t": np.asarray(W3)[sl, :].T.astype(BF16),
            }
        )

    res = run_bass_kernel_spmd(nc, in_maps, core_ids=list(range(N_CORES)))

    out = np.empty((B, G), np.float32)
    for r in range(N_CORES):
        out[:, r * GS : (r + 1) * GS] = res.results[r]["out"].T
    return out


# revision 3
# speedup vs baseline: 1.0049x; 1.0049x over previous
"""Trainium2 Bass kernel for nn_AMLNeuralNetwork3D (dense_mlp).

Strategy: 8-way tensor parallel (column split on output features) for all
three 8192x8192 dense layers; the per-gene local layer shards along the
gene axis (matching the feature split).  After the local layer and after
L1/L2 the per-core feature slices are AllGather'd (concat on partition
axis = gene axis).  L3 slices are returned per-core and assembled on host.

Layout: activations are kept feature-major [features, batch] on chip so a
layer's output layout equals the next layer's input layout (contraction is
over the partition axis on the TensorEngine).  Weights are pre-transposed
on host to [in_features, out_slice] so all DMAs are wide/contiguous.

Prologue: the CC-channel init barrier means no collective can transfer
before ~65us, so the local layer runs on full-batch [128,1024] tiles
(contiguous 256KB x DMAs) and ALL four transition-0 half-gathers are
triggered up-front; the first half lands ~92us and L1 starts there, the
rest land while L1 streams.  PE warmup matmuls bridge the prologue so the
HAM clock-gate stays warm.

Tail: the last chunk of L3 is computed in two feature-half passes so the
final PSUM drain (activations + output DMA) overlaps the second pass.

Compute in bf16 (full-rate on the PE, fp32 PSUM accumulation); measured
L2 rel-err of the full net in bf16 is ~5e-3.
"""

import sys

if "/opt/trn_rl_repo" not in sys.path:
    sys.path.insert(0, "/opt/trn_rl_repo")

import numpy as np
import ml_dtypes

N_CORES = 8
G = 8192          # genes / features
B = 1024          # batch
L = 4             # levels
GS = G // N_CORES # per-core feature slice (1024)
NB = 512          # batch chunk (one PSUM bank at fp32)
NCHUNK = B // NB  # 2
GT = GS // 128    # gene tiles per core slice (8)
KT = G // 128     # contraction tiles (64)

BF16 = ml_dtypes.bfloat16

_compiled = {}

# gathered-feature order when the first AllGather is split into two
# feature halves: [core0 f0:512, core1 f1024:1536, ...] then second halves
_PERM_HALVES = np.concatenate(
    [np.arange(r * GS + a * 512, r * GS + (a + 1) * 512)
     for a in range(2) for r in range(N_CORES)]
)

N_WARMUP = 300


def _build_graph():
    from concourse import bacc, tile
    from concourse.tile_rust import add_dep_helper
    import concourse.mybir as mybir

    fp32 = mybir.dt.float32
    bf16 = mybir.dt.bfloat16
    Relu = mybir.ActivationFunctionType.Relu
    mult = mybir.AluOpType.mult
    add = mybir.AluOpType.add
    bypass = mybir.AluOpType.bypass

    nc = bacc.Bacc(None, target_bir_lowering=False, num_devices=N_CORES)

    # ---- parameters (per-core shards; same graph on all cores) ----
    x_p = nc.declare_dram_parameter("x", [L, GS, B], bf16, isOutput=False)
    # per-feature scalars: cols 0..3 = W_local, 4 = b_local, 5..7 = b1..b3
    scal_p = nc.declare_dram_parameter("scal", [GS, 8], fp32, isOutput=False)
    w_p = [
        nc.declare_dram_parameter(f"w{k}t", [G, GS], bf16, isOutput=False)
        for k in (1, 2, 3)
    ]
    out_p = nc.declare_dram_parameter("out", [GS, B], fp32, isOutput=True)

    rg = [list(range(N_CORES))]

    with tile.TileContext(nc) as tc:
        with (
            tc.tile_pool(name="dram", bufs=1, space="DRAM") as dram,
            tc.tile_pool(name="scal", bufs=GT) as spool,
            tc.tile_pool(name="xin", bufs=10) as xpool,
            tc.tile_pool(name="loc", bufs=6) as lpool,
            tc.tile_pool(name="h0p", bufs=GT) as h0pool,
            tc.tile_pool(name="hin", bufs=32) as hpool,
            tc.tile_pool(name="wblk", bufs=24) as wpool,
            tc.tile_pool(name="hout", bufs=10) as opool,
            tc.tile_pool(name="psum", bufs=8, space="PSUM") as ppool,
        ):
            _gath_space = "Shared"
            # transition-0 bounce buffers: feature-half split per batch chunk
            slc0h = [
                [
                    dram.tile([GS // 2, NB], bf16, name=f"slc0h{j}_{a}",
                              tag=f"slc0h{j}_{a}")
                    for a in range(2)
                ]
                for j in range(NCHUNK)
            ]
            gath0h = [
                [
                    dram.tile(
                        [G // 2, NB], bf16, name=f"gath0h{j}_{a}",
                        tag=f"gath0h{j}_{a}", addr_space=_gath_space,
                    )
                    for a in range(2)
                ]
                for j in range(NCHUNK)
            ]
            # transitions 1,2 (after L1/L2): one AG per batch chunk
            slc = [
                [
                    dram.tile([GS, NB], bf16, name=f"slc_{t}_{j}", tag=f"slc_{t}_{j}")
                    for j in range(NCHUNK)
                ]
                for t in range(2)
            ]
            gath = [
                [
                    dram.tile(
                        [G, NB], bf16, name=f"gath_{t}_{j}", tag=f"gath_{t}_{j}",
                        addr_space=_gath_space,
                    )
                    for j in range(NCHUNK)
                ]
                for t in range(2)
            ]

            # --- PE warmup bridges the prologue (launch barrier + first AG)
            wu_w = spool.tile([128, 128], bf16, name="wu_w", tag="wu_w")
            nc.sync.dma_start(wu_w[:], w_p[0][0:128, 0:128])
            wu_h = spool.tile([128, NB], bf16, name="wu_h", tag="wu_h")
            nc.sync.dma_start(wu_h[:], w_p[0][0:128, 0:NB])
            wu_ps = ppool.tile([128, NB], fp32, name="wu_ps", tag="ps")
            for i in range(N_WARMUP):
                nc.tensor.matmul(
                    wu_ps[:], wu_w[:], wu_h[:],
                    start=(i == 0), stop=(i == N_WARMUP - 1),
                )
            wu_out = spool.tile([128, NB], bf16, name="wu_out", tag="wu_out")
            nc.scalar.activation(
                wu_out[:], wu_ps[:], mybir.ActivationFunctionType.Copy
            )
            wu_dram = dram.tile([128, NB], bf16, name="wu_dram", tag="wu_dram")
            nc.scalar.dma_start(wu_dram[:], wu_out[:])

            # per-feature scalar tiles, persistent
            sc = []
            for gt in range(GT):
                s = spool.tile([128, 8], fp32, name=f"sc{gt}", tag="sc")
                nc.sync.dma_start(s[:], scal_p[gt * 128 : (gt + 1) * 128, :])
                sc.append(s)

            # ---- local layer on full-batch [128, B] tiles (both chunks) ----
            # x DMAs go on the sync ring; activations + slc writes go via the
            # scalar queue so the ring never head-of-line blocks on compute.
            acts = []
            for gt in range(GT):
                xt = []
                for l in range(L):
                    t = xpool.tile([128, B], bf16, name=f"x{gt}_{l}", tag="x")
                    nc.sync.dma_start(t[:], x_p[l, gt * 128 : (gt + 1) * 128, :])
                    xt.append(t)
                acc = lpool.tile([128, B], bf16, name=f"a{gt}_0", tag="acc")
                nc.vector.tensor_scalar(
                    acc[:], xt[0][:], sc[gt][:, 0:1], None, mult
                )
                for l in range(1, L):
                    acc2 = lpool.tile([128, B], bf16, name=f"a{gt}_{l}", tag="acc")
                    nc.vector.scalar_tensor_tensor(
                        acc2[:], xt[l][:], sc[gt][:, l : l + 1], acc[:], mult, add
                    )
                    acc = acc2
                h0 = h0pool.tile([128, B], bf16, name=f"h0_{gt}", tag="h0")
                ai = nc.scalar.activation(h0[:], acc[:], Relu, bias=sc[gt][:, 4:5])
                acts.append(ai)
                a, row = gt // 4, (gt % 4) * 128
                for j in range(NCHUNK):
                    nc.scalar.dma_start(
                        slc0h[j][a][row : row + 128, :],
                        h0[:, j * NB : (j + 1) * NB],
                    )

            # all four transition-0 half-gathers up-front: chunk 0's two
            # halves first (they gate L1), then chunk 1's
            for j in range(NCHUNK):
                for a in range(2):
                    nc.gpsimd.collective_compute(
                        "AllGather", bypass, replica_groups=rg,
                        ins=[slc0h[j][a][:].opt()], outs=[gath0h[j][a][:].opt()],
                    )

            def h_dma(k, j, g, ht):
                if k == 1:
                    hsrc = gath0h[j][g // (KT // 2)]
                    row = (g % (KT // 2)) * 128
                    return nc.sync.dma_start(ht[:], hsrc[row : row + 128, :])
                src = gath[k - 2][j]
                return nc.sync.dma_start(ht[:], src[g * 128 : (g + 1) * 128, :])

            def dense_layer(k, j):
                # k in {1,2,3}; input from gath0h[j] (k==1) or gath[k-2][j];
                # output slice -> slc[k-1][j] (k<3) or out_p (k==3)
                wt = w_p[k - 1]
                ps = [
                    ppool.tile([128, NB], fp32, name=f"ps{k}_{j}_{o}", tag="ps")
                    for o in range(GT)
                ]
                # prefetch the first W blocks ahead of the h stream; for L1
                # chunk 0 hold them behind the local layer's x tiles
                wdmas = {}
                NPREF = 8
                for g in range(NPREF):
                    wb = wpool.tile([128, GS], bf16, name=f"w{k}_{j}_{g}", tag="wblk")
                    wdma = nc.sync.dma_start(wb[:], wt[g * 128 : (g + 1) * 128, :])
                    if k == 1 and j == 0:
                        add_dep_helper(
                            getattr(wdma, "ins", wdma),
                            getattr(acts[3], "ins", acts[3]),
                            reason="x tiles first on HBM",
                        )
                    wdmas[g] = wb
                for g in range(KT):
                    ht = hpool.tile([128, NB], bf16, name=f"h{k}_{j}_{g}", tag="hin")
                    h_dma(k, j, g, ht)
                    if g + NPREF < KT:
                        gq = g + NPREF
                        wb = wpool.tile(
                            [128, GS], bf16, name=f"w{k}_{j}_{gq}", tag="wblk"
                        )
                        nc.sync.dma_start(wb[:], wt[gq * 128 : (gq + 1) * 128, :])
                        wdmas[gq] = wb
                    wb = wdmas.pop(g)
                    for o in range(GT):
                        nc.tensor.matmul(
                            ps[o][:],
                            wb[:, o * 128 : (o + 1) * 128],
                            ht[:],
                            start=(g == 0),
                            stop=(g == KT - 1),
                        )
                for o in range(GT):
                    if k < 3:
                        ot = opool.tile(
                            [128, NB], bf16, name=f"o{k}_{j}_{o}", tag="hout"
                        )
                        nc.scalar.activation(
                            ot[:], ps[o][:], Relu, bias=sc[o][:, 4 + k : 5 + k]
                        )
                        nc.sync.dma_start(
                            slc[k - 1][j][o * 128 : (o + 1) * 128, :], ot[:]
                        )
                    else:
                        ot = opool.tile(
                            [128, NB], fp32, name=f"o{k}_{j}_{o}", tag="outp"
                        )
                        nc.scalar.activation(
                            ot[:], ps[o][:], Relu, bias=sc[o][:, 7:8]
                        )
                        nc.sync.dma_start(
                            out_p[o * 128 : (o + 1) * 128, j * NB : (j + 1) * NB],
                            ot[:],
                        )

            def dense_layer_last(k, j):
                # final chunk: two feature-half passes so the PSUM drain of
                # the first half overlaps the second half's matmuls
                wt = w_p[k - 1]
                for half in range(2):
                    ps = [
                        ppool.tile([128, NB], fp32, name=f"ps{k}_{j}_{half}_{o}",
                                   tag="ps")
                        for o in range(GT // 2)
                    ]
                    col = half * (GS // 2)
                    for g in range(KT):
                        ht = hpool.tile(
                            [128, NB], bf16, name=f"h{k}_{j}_{half}_{g}", tag="hin"
                        )
                        h_dma(k, j, g, ht)
                        wb = wpool.tile(
                            [128, GS // 2], bf16, name=f"w{k}_{j}_{half}_{g}",
                            tag="wblk",
                        )
                        nc.sync.dma_start(
                            wb[:], wt[g * 128 : (g + 1) * 128, col : col + GS // 2]
                        )
                        for o in range(GT // 2):
                            nc.tensor.matmul(
                                ps[o][:],
                                wb[:, o * 128 : (o + 1) * 128],
                                ht[:],
                                start=(g == 0),
                                stop=(g == KT - 1),
                            )
                    for o in range(GT // 2):
                        oo = half * (GT // 2) + o
                        ot = opool.tile(
                            [128, NB], fp32, name=f"o{k}_{j}_{half}_{o}", tag="outp"
                        )
                        nc.scalar.activation(
                            ot[:], ps[o][:], Relu, bias=sc[oo][:, 7:8]
                        )
                        nc.sync.dma_start(
                            out_p[oo * 128 : (oo + 1) * 128,
                                  j * NB : (j + 1) * NB],
                            ot[:],
                        )

            def allgather(t, j):
                nc.gpsimd.collective_compute(
                    "AllGather",
                    bypass,
                    replica_groups=rg,
                    ins=[slc[t - 1][j][:].opt()], outs=[gath[t - 1][j][:].opt()],
                )

            # emission order = desired overlap order
            for k in (1, 2, 3):
                for j in range(NCHUNK):
                    if k == 3 and j == NCHUNK - 1:
                        dense_layer_last(k, j)
                    else:
                        dense_layer(k, j)
                    if k < 3:
                        allgather(k, j)

    nc.compile()
    return nc


def _get_nc():
    if "nc" not in _compiled:
        _compiled["nc"] = _build_graph()
    return _compiled["nc"]


def kernel(x, W_local, b_local, W1, b1, W2, b2, W3, b3):
    from concourse.bass_utils import run_bass_kernel_spmd

    nc = _get_nc()

    x = np.asarray(x)
    in_maps = []
    for r in range(N_CORES):
        sl = slice(r * GS, (r + 1) * GS)
        x_r = x[:, :, sl].transpose(0, 2, 1).astype(BF16)
        scal_r = np.concatenate(
            [
                np.asarray(W_local)[sl, :],
                np.asarray(b_local)[sl, None],
                np.asarray(b1)[sl, None],
                np.asarray(b2)[sl, None],
                np.asarray(b3)[sl, None],
            ],
            axis=1,
        ).astype(np.float32)
        in_maps.append(
            {
                "x": x_r,
                "scal": np.ascontiguousarray(scal_r),
                "w1t": np.asarray(W1)[sl, :].T.astype(BF16)[_PERM_HALVES, :],
                "w2t": np.asarray(W2)[sl, :].T.astype(BF16),
                "w3t": np.asarray(W3)[sl, :].T.astype(BF16),
            }
        )

    res = run_bass_kernel_spmd(nc, in_maps, core_ids=list(range(N_CORES)))

    out = np.empty((B, G), np.float32)
    for r in range(N_CORES):
        out[:, r * GS : (r + 1) * GS] = res.results[r]["out"].T
    return out
